# revision 50
# baseline (speedup 1.0000x reference)
"""Trainium2 Bass kernel for nn_Encoder_29661044146233 (gnn_message_passing).

Approach
--------
The network is linear per output frame, so it folds into a single 22-tap
stride-8 conv (88 -> 66 channels) whose weights are probed on the host in
float64 (see _compose).  The composed map out[t] = A xblk[t-1] + B xblk[t]
+ C xblk[t+1] runs on 8-frame input blocks (704 values zero-padded to
768 = 6*128 so the contraction tiles the full 128-partition dim).

This version runs the matmuls in fp8e4 with the DoubleRow perf mode
(2 K-tiles of 128 per instruction at 0.5 cycles/row), which the TRN2 ISA
permits for M <= 64 and 16-byte-aligned weight pair strides.  The device
therefore computes output rows 0..63; the remaining 2 of the 66 channels
are computed exactly on the host (cheap numpy) and stitched in.

fp8 e4m3 alone is ~3.8% off, so the contraction is error-corrected:

    W x ~= Whi xhi  +  Wlo xhi  +  Whi xlo

with Whi = e4m3(W), Wlo = e4m3(W - Whi), xhi = e4m3(x), xlo = e4m3(x-xhi)
(all at global power-of-2 scales to dodge the e4m3 subnormal floor).  The
two correction terms are band-limited to the high-energy taps (the exact
chunk pairs already present in the main term), which measures ~1.3%
end-to-end vs the 2e-2 tolerance.  Per batch: 9 DR (main) + 6 DR (W corr)
+ 6 DR (x corr) accumulating into one PSUM tile.  The x-corr reuses the
main term's weight slots, so the weight blob stays small.  Boundary-column
edge deltas and the bias ride the host path (exact fp64), not the device.

Scheduling: inputs stream as single-batch DMAs (1101ns supply vs ~1066ns
PE burn per batch; more pieces would saturate the shared HWDGE at 632ns
per DMA).  Batches 0, 14 and 15 run as two column-half chains against
2-piece block-range DMAs: batch 0 to cut the head gate, 14/15 so the
tail critical path ends with the last ~550ns piece plus one 533ns chain
instead of two full batches.  Output stages through 3 SBUF tiles
(precise store deps) with the final 1-batch store split off so the tail
transfer is tiny.  Inputs are two stacked fp8 block arrays (hi, lo) per
batch; out is fp16 (scaled by 2^-15 during the PSUM->SBUF copy, ACT
engine, last chain on DVE).
"""

import os
import sys

for _p in ("/opt/trn_rl_repo", "/root/.axon_site/_ro/trn_rl_repo"):
    if os.path.isdir(_p) and _p not in sys.path:
        sys.path.append(_p)

import numpy as np
import ml_dtypes

TOPOLOGY = [0, 0, 1, 2, 3, 4, 0, 6, 7, 8, 0, 10, 11, 12, 12, 14, 15, 16, 12, 18, 19, 20]
J = 22
POS, OFF = 3, 1
CIN = 88
COUT = 66
MD = 64                   # device-computed output rows (DoubleRow M cap)
NTAP = 22
NEDGE = 15
B, F, T = 128, 2048, 256
NCORES = 8
BL = B // NCORES          # batch per core
NPAIR = BL // 2
UB = 258                  # blocks incl one zero pad each side
BK = 768                  # padded block length (704 data + 64 zero pad)
KC = 6                    # K chunks of 128 per block
XC = UB * KC              # sbuf cols per batch per array (hi or lo)
SW = 2.0 ** 10            # weight scale before e4m3
SX = 2.0 ** 5             # input scale before e4m3
SOUT = 1.0 / (SW * SX)

# DoubleRow slot tables: (slot, u0, c) with u0 the rhs block-window (0=A/x[t-1],
# 1=B/x[t], 2=C/x[t+1]) and c the even base chunk of the (c, c+1) pair.
# Slot 8 is special: C has only 5 nonzero chunks, so instead of a zero
# second half it pairs [Wlo-C2 | Whi-C4] on xhi chunks (2, 4) of the C
# window (stride-2 chunk view) — the W-correction gains chunk C2 for free.
MAIN = [(0, 1, 0), (1, 1, 2), (2, 1, 4),
        (3, 0, 0), (4, 0, 2), (5, 0, 4),
        (6, 2, 0), (7, 2, 2)]
WCORR = [(9, 1, 0), (10, 1, 2), (11, 1, 4),
         (12, 0, 2), (13, 0, 4),
         (14, 2, 0)]
XCORR = [(0, 1, 0), (1, 1, 2), (2, 1, 4),
         (5, 0, 4),
         (6, 2, 0)]
NSLOT = 15
ND = 3                    # edge-delta frames per side (host-applied)


# ---------------------------------------------------------------------------
# host-side weight composition (float64 impulse probing) — unchanged
# ---------------------------------------------------------------------------

def _adj():
    a = np.zeros((J, J), np.float64)
    for i, p in enumerate(TOPOLOGY):
        if i:
            a[p, i] = 1.0
    return a


def _conv_np(z, w, b):
    Bn, Fn, C = z.shape
    zp = np.zeros((Bn, Fn + 2, C), z.dtype)
    zp[:, 1:Fn + 1] = z
    Fo = Fn // 2
    out = np.zeros((Bn, Fo, w.shape[0]), z.dtype)
    for k in range(4):
        out += zp[:, k:k + 2 * Fo:2] @ w[:, :, k].T
    return out + b


def _graph_mat(A, n2n_w, n2n_b, e2n_we, e2n_wn, e2n_b,
               n2e_wn, n2e_we, n2e_b, lin_w, lin_b):
    def apply(z):
        sh = z.shape[:-1]
        zz = z.reshape(-1, J, 4)
        node, edge = zz[..., :POS], zz[..., POS:]
        agg_n = np.einsum('ij,bjc->bic', A, node)
        agg_e = np.einsum('ij,bjc->bic', A, edge)
        f1 = agg_n @ n2n_w + n2n_b
        f2 = agg_e @ e2n_we + node @ e2n_wn + e2n_b
        new_edge = (np.einsum('ji,bjc->bic', A, node) @ n2e_wn
                    + edge @ n2e_we + n2e_b)
        h = np.concatenate([f1, f2], axis=-1) @ lin_w + lin_b
        return np.concatenate([h, new_edge], axis=-1).reshape(*sh, 88)

    g = apply(np.zeros((1, 88)))[0]
    G = apply(np.eye(88)) - g
    return G.T, g


def _compose(P):
    A = _adj()
    P64 = {k: np.asarray(v, np.float64) for k, v in P.items()}
    gnames = ('n2n_w', 'n2n_b', 'e2n_we', 'e2n_wn', 'e2n_b',
              'n2e_wn', 'n2e_we', 'n2e_b', 'lin_w', 'lin_b')
    G1, g1 = _graph_mat(A, *[P64['g1_' + s] for s in gnames])
    G2, g2 = _graph_mat(A, *[P64['g2_' + s] for s in gnames])
    keep = np.array([4 * j + c for j in range(J) for c in range(POS)])

    def pipeline(x88):
        y = _conv_np(x88, P64['conv1_w'], P64['conv1_b'])
        y = y @ G1.T + g1
        y = _conv_np(y, P64['conv2_w'], P64['conv2_b'])
        y = y @ G2.T + g2
        y = _conv_np(y, P64['conv3_w'], P64['conv3_b'])
        return y[..., keep]

    Fp = 256
    Tp = Fp // 8
    zb = pipeline(np.zeros((1, Fp, 88)))[0]
    bint, bl, br = zb[Tp // 2], zb[0], zb[Tp - 1]

    mid = Fp // 2
    probes = np.zeros((8 * 88, Fp, 88))
    for r in range(8):
        for ic in range(88):
            probes[r * 88 + ic, mid + r, ic] = 1.0
    resp = pipeline(probes) - zb
    wint = np.zeros((NTAP, COUT, CIN))
    for r in range(8):
        for t in range(Tp):
            m = (mid + r) - 8 * t + 7
            if 0 <= m < NTAP:
                wint[m] = resp[r * 88:(r + 1) * 88, t, :].T

    probes = np.zeros((NEDGE * 88, Fp, 88))
    for f in range(NEDGE):
        for ic in range(88):
            probes[f * 88 + ic, f, ic] = 1.0
    resp = pipeline(probes) - zb
    wl = np.stack([resp[f * 88:(f + 1) * 88, 0, :].T for f in range(NEDGE)])

    probes = np.zeros((NEDGE * 88, Fp, 88))
    for f in range(NEDGE):
        for ic in range(88):
            probes[f * 88 + ic, Fp - NEDGE + f, ic] = 1.0
    resp = pipeline(probes) - zb
    wr = np.stack([resp[f * 88:(f + 1) * 88, Tp - 1, :].T for f in range(NEDGE)])

    return dict(wint=wint, bint=bint, wl=wl, wr=wr, bl=bl, br=br)


# ---------------------------------------------------------------------------
# device program (built/compiled once, reused across calls)
# ---------------------------------------------------------------------------

_STATE = {}

DEFAULT_OPTS = dict(
    warm_n=17,          # warm-up matmul count (bridge p-state ramp)
    memset_cols=2,
    xs_bufs=5,
    tail_cols=32,       # final col-chain width (T = no split)
)


def _build_device(opts=None):
    import concourse.bass as bass  # noqa: F401
    import concourse.tile as tile
    from concourse import bacc, mybir

    o_ = dict(DEFAULT_OPTS)
    if opts:
        o_.update(opts)
    f32 = mybir.dt.float32
    f16 = mybir.dt.float16
    f8 = mybir.dt.float8e4
    DR = mybir.MatmulPerfMode.DoubleRow
    nc = bacc.Bacc("TRN2", target_bir_lowering=False, debug=False,
                   num_devices=NCORES)

    wsb_d = nc.dram_tensor("wsb", [128, NSLOT * 128], f8, kind="ExternalInput")
    xh_d = nc.dram_tensor("xh", [NPAIR, 128, 2, 2 * XC], f8, kind="ExternalInput")
    out_d = nc.dram_tensor("out", [MD, BL, T], f16, kind="ExternalOutput")

    with tile.TileContext(nc) as tc:
        with (
            tc.tile_pool(name="consts", bufs=1) as consts,
            tc.tile_pool(name="xs", bufs=o_["xs_bufs"]) as xspool,
            tc.tile_pool(name="ps1", bufs=4, space="PSUM") as ps1pool,
            tc.tile_pool(name="warm", bufs=1, space="PSUM") as warmpool,
            tc.tile_pool(name="ob", bufs=1) as opool,
        ):
            # PE warm-up: dummy bf16 matmuls on scratch, no DMA deps, to
            # bridge the ~3us p-state ramp while the first DMAs stream.
            bf16 = mybir.dt.bfloat16
            scratch = consts.tile([CIN, 162], f32)
            if o_["memset_cols"]:
                nc.vector.memset(scratch[:, 0:o_["memset_cols"]], 0.0)
            s16 = scratch[:].bitcast(bf16)
            wps = warmpool.tile([COUT, 256], f32)
            for _ in range(o_["warm_n"]):
                nc.tensor.matmul(wps[:], lhsT=s16[:, 0:COUT],
                                 rhs=s16[:, 66:322], start=True, stop=True)

            # DMA order: weight blob + batch-0 input first, then edge blob,
            # then per-pair input streams.
            # batch-0 input split by block range so the first column-half
            # conv can start ~550ns before the rest of batch 0 lands
            H0 = (128 + 2) * KC
            wsb = consts.tile([128, NSLOT, 2, MD], f8)
            x0 = xspool.tile([128, 2, 2, XC], f8)
            x0v_d = xh_d[0][:, 0].rearrange("p (a x) -> p a x", a=2)
            nc.sync.dma_start(out=wsb[:], in_=wsb_d[:])
            nc.sync.dma_start(out=x0[:, 0, :, 0:H0], in_=x0v_d[:, :, 0:H0])
            nc.sync.dma_start(out=x0[:, 0, :, H0:], in_=x0v_d[:, :, H0:])
            nc.sync.dma_start(out=x0[:, 1], in_=xh_d[0][:, 1].rearrange(
                "p (a x) -> p a x", a=2))

            def xpair(p):
                # two single-batch DMAs: supply (1101ns/batch) then tracks
                # just ahead of the PE burn rate (~1066ns/batch); more
                # pieces would saturate the shared HWDGE (632ns per DMA)
                xt = xspool.tile([128, 2, 2, XC], f8)
                for b in range(2):
                    nc.sync.dma_start(
                        out=xt[:, b],
                        in_=xh_d[p][:, b].rearrange("p (a x) -> p a x", a=2))
                return xt

            # split output staging into 3 tiles so each store DMA depends
            # only on its own batches (coarse tile deps otherwise park the
            # mid store behind the final batch)
            obA = opool.tile([MD, 7, T], f16)
            obB = opool.tile([MD, 8, T], f16)
            obC = opool.tile([MD, 1, T], f16)

            def obsel(boff):
                if boff < 7:
                    return obA, boff
                if boff < 15:
                    return obB, boff - 7
                return obC, 0

            def conv(xt, b0, boff, c0=0, nc_=T, copy_eng="act"):
                # one batch; out col window [c0, c0+nc_).  Boundary-column
                # edge deltas are applied on the host.
                xv = xt[:].rearrange("p b a (u s) -> p b a u s", s=KC)
                xv2 = xt[:].rearrange("p b a (u s2 s) -> p b a u s2 s",
                                      s2=KC // 2, s=2)
                t1 = ps1pool.tile([MD, nc_], f32)
                nmm = len(MAIN) + 1 + len(WCORR) + len(XCORR)
                k = 0

                def rhs(a, u0, c):
                    return xv[:, b0, a, u0 + c0:u0 + c0 + nc_, c:c + 2] \
                        .rearrange("p u s -> p s u")

                for table, a in ((MAIN, 0), (WCORR, 0), (XCORR, 1)):
                    for slot, u0, c in table:
                        k += 1
                        nc.tensor.matmul(
                            t1[:], lhsT=wsb[:, slot], rhs=rhs(a, u0, c),
                            start=(k == 1), stop=(k == nmm), perf_mode=DR)
                    if table is MAIN:
                        # slot 8: C-window xhi chunks (2, 4), stride 2
                        k += 1
                        rhs8 = xv2[:, b0, 0, 2 + c0:2 + c0 + nc_, 1:3, 0] \
                            .rearrange("p u s -> p s u")
                        nc.tensor.matmul(
                            t1[:], lhsT=wsb[:, 8], rhs=rhs8,
                            start=False, stop=False, perf_mode=DR)

                ot, oi = obsel(boff)
                o = ot[:, oi:oi + 1, c0:c0 + nc_].rearrange("m b n -> m (b n)")
                if copy_eng == "dve":
                    nc.vector.tensor_scalar_mul(o, t1[:], SOUT)
                else:
                    nc.scalar.activation(
                        o, t1[:], mybir.ActivationFunctionType.Identity,
                        bias=0.0, scale=SOUT)

            x1 = xpair(1)
            conv(x0, 0, 0, 0, 128)
            conv(x0, 0, 0, 128, 128)
            conv(x0, 1, 1)
            conv(x1, 0, 2)
            conv(x1, 1, 3)
            for p in range(2, NPAIR - 1):
                xt = xpair(p)
                conv(xt, 0, 2 * p)
                conv(xt, 1, 2 * p + 1)
            # last pair: batch 14 whole, batch 15 as two block-range
            # pieces so its first column-half chain runs while the second
            # piece is still streaming
            xl = xspool.tile([128, 2, 2, XC], f8)
            xlv_d = xh_d[NPAIR - 1]
            for b in range(2):
                xb_d = xlv_d[:, b].rearrange("p (a x) -> p a x", a=2)
                nc.sync.dma_start(out=xl[:, b, :, 0:H0], in_=xb_d[:, :, 0:H0])
                nc.sync.dma_start(out=xl[:, b, :, H0:], in_=xb_d[:, :, H0:])
            # obA store issued on the same in-order queue after the last
            # input fetch so its transfer cannot delay those batches
            nc.sync.dma_start(out=out_d[:, 0:7, :], in_=obA[:])
            conv(xl, 0, BL - 2, 0, 128)
            conv(xl, 0, BL - 2, 128, 128)
            nc.sync.dma_start(out=out_d[:, 7:15, :], in_=obB[:])
            conv(xl, 1, BL - 1, 0, 128)
            conv(xl, 1, BL - 1, 128, 128, copy_eng="dve")
            nc.sync.dma_start(out=out_d[:, 15:BL, :], in_=obC[:])

    nc.compile()
    return nc


def _get_state():
    if "nc" not in _STATE:
        _STATE["nc"] = _build_device()
    return _STATE["nc"]


# ---------------------------------------------------------------------------
# host packing
# ---------------------------------------------------------------------------

def _fp8(v):
    return np.asarray(v, dtype=ml_dtypes.float8_e4m3fn)


def _host_pack(C, x88):
    """Marshal composed weights + inputs into the device tensors."""
    wint = C["wint"]

    Am = np.zeros((COUT, BK))
    Bm = np.zeros((COUT, BK))
    Cm = np.zeros((COUT, BK))
    for m in range(NTAP):
        if m < 7:
            Am[:, 88 * (m + 1):88 * (m + 2)] = wint[m]
        elif m < 15:
            Bm[:, 88 * (m - 7):88 * (m - 6)] = wint[m]
        else:
            Cm[:, 88 * (m - 15):88 * (m - 14)] = wint[m]
    maps = {0: Am, 1: Bm, 2: Cm}
    hi = {}
    lo = {}
    for u0, M in maps.items():
        h = _fp8(M * SW)
        hi[u0] = h
        lo[u0] = _fp8(M * SW - h.astype(np.float64))

    wsb = np.zeros((128, NSLOT, 2, MD), ml_dtypes.float8_e4m3fn)
    for slot, u0, c in MAIN:
        for j in range(2):
            cc = c + j
            if cc < KC and not (u0 == 2 and cc == 5):
                wsb[:, slot, j, :] = hi[u0][:MD, 128 * cc:128 * cc + 128].T
    for slot, u0, c in WCORR:
        for j in range(2):
            cc = c + j
            if cc < KC and not (u0 == 2 and cc == 5):
                wsb[:, slot, j, :] = lo[u0][:MD, 128 * cc:128 * cc + 128].T
    # slot 8: [Wlo-C2 | Whi-C4] (rhs = C-window xhi chunks (2, 4))
    wsb[:, 8, 0, :] = lo[2][:MD, 256:384].T
    wsb[:, 8, 1, :] = hi[2][:MD, 512:640].T
    wsb = wsb.reshape(128, NSLOT * 128)

    # input marshalling: [B, F, 88] -> scaled hi/lo padded blocks
    xb = np.zeros((B, UB, BK))
    xb[:, 1:257, :704] = x88.reshape(B, T, 704) * SX
    xhi = _fp8(xb)
    xlo = _fp8(xb - xhi.astype(np.float64))
    xs = np.stack([xhi, xlo], axis=1)        # [B, 2, UB, BK]
    xh = np.ascontiguousarray(
        xs.reshape(B // 2, 2, 2, UB, KC, 128).transpose(0, 5, 1, 2, 3, 4)
    ).reshape(B // 2, 128, 2, 2 * XC)

    return wsb, xh


def _host_tail(C, x88):
    """Exact host computation of output channels MD..66 plus the bias
    terms (all-zero for the given inputs, kept for generality)."""
    wint, wl, wr = C["wint"], C["wl"], C["wr"]
    xp = np.zeros((B, F + 16, CIN))
    xp[:, 7:7 + F] = x88
    h2 = np.zeros((B, T, COUT - MD))
    for m in range(NTAP):
        h2 += xp[:, m:m + 8 * T:8] @ wint[m, MD:COUT].T
    dwl = wl[:3] - wint[7:10]
    dwr = wr[12:15] - wint[12:15]
    for e in range(ND):
        h2[:, 0] += x88[:, e] @ dwl[e, MD:COUT].T
        h2[:, T - 1] += x88[:, F - ND + e] @ dwr[e, MD:COUT].T
    h2 += C["bint"][MD:COUT]
    h2[:, 0] += (C["bl"] - C["bint"])[MD:COUT]
    h2[:, T - 1] += (C["br"] - C["bint"])[MD:COUT]
    return h2


# ---------------------------------------------------------------------------
# entry point
# ---------------------------------------------------------------------------

def _kernel_impl(**inputs):
    from concourse.bass_utils import run_bass_kernel_spmd

    P = {k: np.asarray(v) for k, v in inputs.items()}
    inp = P.pop("input").astype(np.float64, copy=False)
    off = P.pop("offset").astype(np.float64, copy=False)
    x88 = np.concatenate([inp, off], -1).reshape(B, F, CIN)

    C = _compose(P)
    wsb, xh = _host_pack(C, x88)
    h2 = _host_tail(C, x88)

    in_maps = []
    for c in range(NCORES):
        in_maps.append({
            "wsb": wsb,
            "xh": xh[c * NPAIR:(c + 1) * NPAIR],
        })

    nc = _get_state()
    res = run_bass_kernel_spmd(nc, in_maps, core_ids=list(range(NCORES)))

    out = np.empty((B, T, COUT), np.float32)
    for c in range(NCORES):
        o = res.results[c]["out"].astype(np.float32)             # [64, BL, 256]
        out[c * BL:(c + 1) * BL, :, :MD] = o.transpose(1, 2, 0)
    # bias + boundary-column edge deltas for the device channels (exact,
    # host-side; the device computes the interior approximation only)
    wint, wl, wr = C["wint"], C["wl"], C["wr"]
    out[:, :, :MD] += C["bint"][:MD]
    out[:, 0, :MD] += (C["bl"] - C["bint"])[:MD]
    out[:, T - 1, :MD] += (C["br"] - C["bint"])[:MD]
    dwl = wl[:3] - wint[7:10]
    dwr = wr[12:15] - wint[12:15]
    for e in range(ND):
        out[:, 0, :MD] += x88[:, e] @ dwl[e, :MD].T
        out[:, T - 1, :MD] += x88[:, F - ND + e] @ dwr[e, :MD].T
    out[:, :, MD:] = h2
    return out.reshape(B, T, J, POS)


def _subproc_main(in_path, out_path):
    with open(in_path, "rb") as f:
        import pickle
        inputs = pickle.load(f)
    np.save(out_path, _kernel_impl(**inputs))


def kernel(**inputs):
    """Entry point. The very first execution of a freshly compiled NEFF
    occasionally kills the device session (NRT_EXEC_UNIT_UNRECOVERABLE);
    a rerun in a fresh process reliably succeeds (the compile cache makes
    it cheap). So: try in-process, fall back to fresh subprocesses."""
    if not _STATE.get("dead"):
        try:
            return _kernel_impl(**inputs)
        except Exception:  # noqa: BLE001
            _STATE["dead"] = True  # this process's device session is gone

    import pickle
    import subprocess
    import tempfile

    kdir = os.path.dirname(os.path.abspath(__file__))
    last_err = None
    for _ in range(3):
        with tempfile.TemporaryDirectory() as td:
            ip = os.path.join(td, "in.pkl")
            op = os.path.join(td, "out.npy")
            with open(ip, "wb") as f:
                pickle.dump({k: np.asarray(v) for k, v in inputs.items()}, f,
                            protocol=4)
            code = (
                "import sys; sys.path.insert(0, {kd!r}); import kernel; "
                "kernel._subproc_main({ip!r}, {op!r})"
            ).format(kd=kdir, ip=ip, op=op)
            r = subprocess.run([sys.executable, "-c", code],
                               capture_output=True, text=True)
            if r.returncode == 0 and os.path.exists(op):
                return np.load(op)
            last_err = r.stderr[-2000:] if r.stderr else f"rc={r.returncode}"
    raise RuntimeError(f"kernel subprocess retries exhausted: {last_err}")


# revision 51
# speedup vs baseline: 1.0045x; 1.0045x over previous
"""Trainium2 Bass kernel for nn_Encoder_29661044146233 (gnn_message_passing).

Approach
--------
The network is linear per output frame, so it folds into a single 22-tap
stride-8 conv (88 -> 66 channels) whose weights are probed on the host in
float64 (see _compose).  The composed map out[t] = A xblk[t-1] + B xblk[t]
+ C xblk[t+1] runs on 8-frame input blocks (704 values zero-padded to
768 = 6*128 so the contraction tiles the full 128-partition dim).

This version runs the matmuls in fp8e4 with the DoubleRow perf mode
(2 K-tiles of 128 per instruction at 0.5 cycles/row), which the TRN2 ISA
permits for M <= 64 and 16-byte-aligned weight pair strides.  The device
therefore computes output rows 0..63; the remaining 2 of the 66 channels
are computed exactly on the host (cheap numpy) and stitched in.

fp8 e4m3 alone is ~3.8% off, so the contraction is error-corrected:

    W x ~= Whi xhi  +  Wlo xhi  +  Whi xlo

with Whi = e4m3(W), Wlo = e4m3(W - Whi), xhi = e4m3(x), xlo = e4m3(x-xhi)
(all at global power-of-2 scales to dodge the e4m3 subnormal floor).  The
two correction terms are band-limited to the high-energy taps (the exact
chunk pairs already present in the main term), which measures ~1.3%
end-to-end vs the 2e-2 tolerance.  Per batch: 9 DR (main) + 6 DR (W corr)
+ 6 DR (x corr) accumulating into one PSUM tile.  The x-corr reuses the
main term's weight slots, so the weight blob stays small.  Boundary-column
edge deltas and the bias ride the host path (exact fp64), not the device.

Scheduling: inputs stream as single-batch DMAs (1101ns supply vs ~1066ns
PE burn per batch; more pieces would saturate the shared HWDGE at 632ns
per DMA).  Batches 0, 14 and 15 run as two column-half chains against
2-piece block-range DMAs: batch 0 to cut the head gate, 14/15 so the
tail critical path ends with the last ~550ns piece plus one 533ns chain
instead of two full batches.  Output stages through 3 SBUF tiles
(precise store deps) with the final 1-batch store split off so the tail
transfer is tiny.  Inputs are two stacked fp8 block arrays (hi, lo) per
batch; out is fp16 (scaled by 2^-15 during the PSUM->SBUF copy, ACT
engine, last chain on DVE).
"""

import os
import sys

for _p in ("/opt/trn_rl_repo", "/root/.axon_site/_ro/trn_rl_repo"):
    if os.path.isdir(_p) and _p not in sys.path:
        sys.path.append(_p)

import numpy as np
import ml_dtypes

TOPOLOGY = [0, 0, 1, 2, 3, 4, 0, 6, 7, 8, 0, 10, 11, 12, 12, 14, 15, 16, 12, 18, 19, 20]
J = 22
POS, OFF = 3, 1
CIN = 88
COUT = 66
MD = 64                   # device-computed output rows (DoubleRow M cap)
NTAP = 22
NEDGE = 15
B, F, T = 128, 2048, 256
NCORES = 8
BL = B // NCORES          # batch per core
NPAIR = BL // 2
UB = 258                  # blocks incl one zero pad each side
BK = 768                  # padded block length (704 data + 64 zero pad)
KC = 6                    # K chunks of 128 per block
XC = UB * KC              # sbuf cols per batch per array (hi or lo)
SW = 2.0 ** 10            # weight scale before e4m3
SX = 2.0 ** 5             # input scale before e4m3
SOUT = 1.0 / (SW * SX)

# DoubleRow slot tables: (slot, u0, c) with u0 the rhs block-window (0=A/x[t-1],
# 1=B/x[t], 2=C/x[t+1]) and c the even base chunk of the (c, c+1) pair.
# Slot 8 is special: C has only 5 nonzero chunks, so instead of a zero
# second half it pairs [Wlo-C2 | Whi-C4] on xhi chunks (2, 4) of the C
# window (stride-2 chunk view) — the W-correction gains chunk C2 for free.
MAIN = [(0, 1, 0), (1, 1, 2), (2, 1, 4),
        (3, 0, 0), (4, 0, 2), (5, 0, 4),
        (6, 2, 0), (7, 2, 2)]
WCORR = [(9, 1, 0), (10, 1, 2), (11, 1, 4),
         (12, 0, 2), (13, 0, 4),
         (14, 2, 0)]
XCORR = [(0, 1, 0), (1, 1, 2), (2, 1, 4),
         (5, 0, 4),
         (6, 2, 0)]
NSLOT = 15
ND = 3                    # edge-delta frames per side (host-applied)


# ---------------------------------------------------------------------------
# host-side weight composition (float64 impulse probing) — unchanged
# ---------------------------------------------------------------------------

def _adj():
    a = np.zeros((J, J), np.float64)
    for i, p in enumerate(TOPOLOGY):
        if i:
            a[p, i] = 1.0
    return a


def _conv_np(z, w, b):
    Bn, Fn, C = z.shape
    zp = np.zeros((Bn, Fn + 2, C), z.dtype)
    zp[:, 1:Fn + 1] = z
    Fo = Fn // 2
    out = np.zeros((Bn, Fo, w.shape[0]), z.dtype)
    for k in range(4):
        out += zp[:, k:k + 2 * Fo:2] @ w[:, :, k].T
    return out + b


def _graph_mat(A, n2n_w, n2n_b, e2n_we, e2n_wn, e2n_b,
               n2e_wn, n2e_we, n2e_b, lin_w, lin_b):
    def apply(z):
        sh = z.shape[:-1]
        zz = z.reshape(-1, J, 4)
        node, edge = zz[..., :POS], zz[..., POS:]
        agg_n = np.einsum('ij,bjc->bic', A, node)
        agg_e = np.einsum('ij,bjc->bic', A, edge)
        f1 = agg_n @ n2n_w + n2n_b
        f2 = agg_e @ e2n_we + node @ e2n_wn + e2n_b
        new_edge = (np.einsum('ji,bjc->bic', A, node) @ n2e_wn
                    + edge @ n2e_we + n2e_b)
        h = np.concatenate([f1, f2], axis=-1) @ lin_w + lin_b
        return np.concatenate([h, new_edge], axis=-1).reshape(*sh, 88)

    g = apply(np.zeros((1, 88)))[0]
    G = apply(np.eye(88)) - g
    return G.T, g


def _compose(P):
    A = _adj()
    P64 = {k: np.asarray(v, np.float64) for k, v in P.items()}
    gnames = ('n2n_w', 'n2n_b', 'e2n_we', 'e2n_wn', 'e2n_b',
              'n2e_wn', 'n2e_we', 'n2e_b', 'lin_w', 'lin_b')
    G1, g1 = _graph_mat(A, *[P64['g1_' + s] for s in gnames])
    G2, g2 = _graph_mat(A, *[P64['g2_' + s] for s in gnames])
    keep = np.array([4 * j + c for j in range(J) for c in range(POS)])

    def pipeline(x88):
        y = _conv_np(x88, P64['conv1_w'], P64['conv1_b'])
        y = y @ G1.T + g1
        y = _conv_np(y, P64['conv2_w'], P64['conv2_b'])
        y = y @ G2.T + g2
        y = _conv_np(y, P64['conv3_w'], P64['conv3_b'])
        return y[..., keep]

    Fp = 256
    Tp = Fp // 8
    zb = pipeline(np.zeros((1, Fp, 88)))[0]
    bint, bl, br = zb[Tp // 2], zb[0], zb[Tp - 1]

    mid = Fp // 2
    probes = np.zeros((8 * 88, Fp, 88))
    for r in range(8):
        for ic in range(88):
            probes[r * 88 + ic, mid + r, ic] = 1.0
    resp = pipeline(probes) - zb
    wint = np.zeros((NTAP, COUT, CIN))
    for r in range(8):
        for t in range(Tp):
            m = (mid + r) - 8 * t + 7
            if 0 <= m < NTAP:
                wint[m] = resp[r * 88:(r + 1) * 88, t, :].T

    probes = np.zeros((NEDGE * 88, Fp, 88))
    for f in range(NEDGE):
        for ic in range(88):
            probes[f * 88 + ic, f, ic] = 1.0
    resp = pipeline(probes) - zb
    wl = np.stack([resp[f * 88:(f + 1) * 88, 0, :].T for f in range(NEDGE)])

    probes = np.zeros((NEDGE * 88, Fp, 88))
    for f in range(NEDGE):
        for ic in range(88):
            probes[f * 88 + ic, Fp - NEDGE + f, ic] = 1.0
    resp = pipeline(probes) - zb
    wr = np.stack([resp[f * 88:(f + 1) * 88, Tp - 1, :].T for f in range(NEDGE)])

    return dict(wint=wint, bint=bint, wl=wl, wr=wr, bl=bl, br=br)


# ---------------------------------------------------------------------------
# device program (built/compiled once, reused across calls)
# ---------------------------------------------------------------------------

_STATE = {}

DEFAULT_OPTS = dict(
    warm_n=17,          # warm-up matmul count (bridge p-state ramp)
    memset_cols=2,
    xs_bufs=5,
    tail_cols=32,       # final col-chain width (T = no split)
)


def _build_device(opts=None):
    import concourse.bass as bass  # noqa: F401
    import concourse.tile as tile
    from concourse import bacc, mybir

    o_ = dict(DEFAULT_OPTS)
    if opts:
        o_.update(opts)
    f32 = mybir.dt.float32
    f16 = mybir.dt.float16
    f8 = mybir.dt.float8e4
    DR = mybir.MatmulPerfMode.DoubleRow
    nc = bacc.Bacc("TRN2", target_bir_lowering=False, debug=False,
                   num_devices=NCORES)

    wsb_d = nc.dram_tensor("wsb", [128, NSLOT * 128], f8, kind="ExternalInput")
    xh_d = nc.dram_tensor("xh", [NPAIR, 128, 2, 2 * XC], f8, kind="ExternalInput")
    out_d = nc.dram_tensor("out", [MD, BL, T], f16, kind="ExternalOutput")

    with tile.TileContext(nc) as tc:
        with (
            tc.tile_pool(name="consts", bufs=1) as consts,
            tc.tile_pool(name="xs", bufs=o_["xs_bufs"]) as xspool,
            tc.tile_pool(name="ps1", bufs=4, space="PSUM") as ps1pool,
            tc.tile_pool(name="warm", bufs=1, space="PSUM") as warmpool,
            tc.tile_pool(name="ob", bufs=1) as opool,
        ):
            # PE warm-up: dummy bf16 matmuls on scratch, no DMA deps, to
            # bridge the ~3us p-state ramp while the first DMAs stream.
            bf16 = mybir.dt.bfloat16
            scratch = consts.tile([CIN, 162], f32)
            if o_["memset_cols"]:
                nc.vector.memset(scratch[:, 0:o_["memset_cols"]], 0.0)
            s16 = scratch[:].bitcast(bf16)
            wps = warmpool.tile([COUT, 256], f32)
            for _ in range(o_["warm_n"]):
                nc.tensor.matmul(wps[:], lhsT=s16[:, 0:COUT],
                                 rhs=s16[:, 66:322], start=True, stop=True)

            # DMA order: weight blob + batch-0 input first, then edge blob,
            # then per-pair input streams.
            # batch-0 input split by block range so the first column-half
            # conv can start ~550ns before the rest of batch 0 lands
            H0 = (128 + 2) * KC
            wsb = consts.tile([128, NSLOT, 2, MD], f8)
            x0 = xspool.tile([128, 2, 2, XC], f8)
            x0v_d = xh_d[0][:, 0].rearrange("p (a x) -> p a x", a=2)
            nc.gpsimd.dma_start(out=wsb[:], in_=wsb_d[:])
            nc.sync.dma_start(out=x0[:, 0, :, 0:H0], in_=x0v_d[:, :, 0:H0])
            nc.sync.dma_start(out=x0[:, 0, :, H0:], in_=x0v_d[:, :, H0:])
            nc.sync.dma_start(out=x0[:, 1], in_=xh_d[0][:, 1].rearrange(
                "p (a x) -> p a x", a=2))

            def xpair(p):
                # two single-batch DMAs: supply (1101ns/batch) then tracks
                # just ahead of the PE burn rate (~1066ns/batch); more
                # pieces would saturate the shared HWDGE (632ns per DMA)
                xt = xspool.tile([128, 2, 2, XC], f8)
                for b in range(2):
                    nc.sync.dma_start(
                        out=xt[:, b],
                        in_=xh_d[p][:, b].rearrange("p (a x) -> p a x", a=2))
                return xt

            # split output staging into 3 tiles so each store DMA depends
            # only on its own batches (coarse tile deps otherwise park the
            # mid store behind the final batch)
            obA = opool.tile([MD, 7, T], f16)
            obB = opool.tile([MD, 8, T], f16)
            obC = opool.tile([MD, 1, T], f16)

            def obsel(boff):
                if boff < 7:
                    return obA, boff
                if boff < 15:
                    return obB, boff - 7
                return obC, 0

            def conv(xt, b0, boff, c0=0, nc_=T, copy_eng="act"):
                # one batch; out col window [c0, c0+nc_).  Boundary-column
                # edge deltas are applied on the host.
                xv = xt[:].rearrange("p b a (u s) -> p b a u s", s=KC)
                xv2 = xt[:].rearrange("p b a (u s2 s) -> p b a u s2 s",
                                      s2=KC // 2, s=2)
                t1 = ps1pool.tile([MD, nc_], f32)
                nmm = len(MAIN) + 1 + len(WCORR) + len(XCORR)
                k = 0

                def rhs(a, u0, c):
                    return xv[:, b0, a, u0 + c0:u0 + c0 + nc_, c:c + 2] \
                        .rearrange("p u s -> p s u")

                for table, a in ((MAIN, 0), (WCORR, 0), (XCORR, 1)):
                    for slot, u0, c in table:
                        k += 1
                        nc.tensor.matmul(
                            t1[:], lhsT=wsb[:, slot], rhs=rhs(a, u0, c),
                            start=(k == 1), stop=(k == nmm), perf_mode=DR)
                    if table is MAIN:
                        # slot 8: C-window xhi chunks (2, 4), stride 2
                        k += 1
                        rhs8 = xv2[:, b0, 0, 2 + c0:2 + c0 + nc_, 1:3, 0] \
                            .rearrange("p u s -> p s u")
                        nc.tensor.matmul(
                            t1[:], lhsT=wsb[:, 8], rhs=rhs8,
                            start=False, stop=False, perf_mode=DR)

                ot, oi = obsel(boff)
                o = ot[:, oi:oi + 1, c0:c0 + nc_].rearrange("m b n -> m (b n)")
                if copy_eng == "dve":
                    nc.vector.tensor_scalar_mul(o, t1[:], SOUT)
                else:
                    nc.scalar.activation(
                        o, t1[:], mybir.ActivationFunctionType.Identity,
                        bias=0.0, scale=SOUT)

            x1 = xpair(1)
            conv(x0, 0, 0, 0, 128)
            conv(x0, 0, 0, 128, 128)
            conv(x0, 1, 1)
            conv(x1, 0, 2)
            conv(x1, 1, 3)
            for p in range(2, NPAIR - 1):
                xt = xpair(p)
                conv(xt, 0, 2 * p)
                conv(xt, 1, 2 * p + 1)
            # last pair: batch 14 whole, batch 15 as two block-range
            # pieces so its first column-half chain runs while the second
            # piece is still streaming
            xl = xspool.tile([128, 2, 2, XC], f8)
            xlv_d = xh_d[NPAIR - 1]
            for b in range(2):
                xb_d = xlv_d[:, b].rearrange("p (a x) -> p a x", a=2)
                nc.sync.dma_start(out=xl[:, b, :, 0:H0], in_=xb_d[:, :, 0:H0])
                nc.sync.dma_start(out=xl[:, b, :, H0:], in_=xb_d[:, :, H0:])
            # obA store issued on the same in-order queue after the last
            # input fetch so its transfer cannot delay those batches
            nc.sync.dma_start(out=out_d[:, 0:7, :], in_=obA[:])
            conv(xl, 0, BL - 2, 0, 128)
            conv(xl, 0, BL - 2, 128, 128)
            nc.sync.dma_start(out=out_d[:, 7:15, :], in_=obB[:])
            conv(xl, 1, BL - 1, 0, 128)
            conv(xl, 1, BL - 1, 128, 128, copy_eng="dve")
            nc.sync.dma_start(out=out_d[:, 15:BL, :], in_=obC[:])

    nc.compile()
    return nc


def _get_state():
    if "nc" not in _STATE:
        _STATE["nc"] = _build_device()
    return _STATE["nc"]


# ---------------------------------------------------------------------------
# host packing
# ---------------------------------------------------------------------------

def _fp8(v):
    return np.asarray(v, dtype=ml_dtypes.float8_e4m3fn)


def _host_pack(C, x88):
    """Marshal composed weights + inputs into the device tensors."""
    wint = C["wint"]

    Am = np.zeros((COUT, BK))
    Bm = np.zeros((COUT, BK))
    Cm = np.zeros((COUT, BK))
    for m in range(NTAP):
        if m < 7:
            Am[:, 88 * (m + 1):88 * (m + 2)] = wint[m]
        elif m < 15:
            Bm[:, 88 * (m - 7):88 * (m - 6)] = wint[m]
        else:
            Cm[:, 88 * (m - 15):88 * (m - 14)] = wint[m]
    maps = {0: Am, 1: Bm, 2: Cm}
    hi = {}
    lo = {}
    for u0, M in maps.items():
        h = _fp8(M * SW)
        hi[u0] = h
        lo[u0] = _fp8(M * SW - h.astype(np.float64))

    wsb = np.zeros((128, NSLOT, 2, MD), ml_dtypes.float8_e4m3fn)
    for slot, u0, c in MAIN:
        for j in range(2):
            cc = c + j
            if cc < KC and not (u0 == 2 and cc == 5):
                wsb[:, slot, j, :] = hi[u0][:MD, 128 * cc:128 * cc + 128].T
    for slot, u0, c in WCORR:
        for j in range(2):
            cc = c + j
            if cc < KC and not (u0 == 2 and cc == 5):
                wsb[:, slot, j, :] = lo[u0][:MD, 128 * cc:128 * cc + 128].T
    # slot 8: [Wlo-C2 | Whi-C4] (rhs = C-window xhi chunks (2, 4))
    wsb[:, 8, 0, :] = lo[2][:MD, 256:384].T
    wsb[:, 8, 1, :] = hi[2][:MD, 512:640].T
    wsb = wsb.reshape(128, NSLOT * 128)

    # input marshalling: [B, F, 88] -> scaled hi/lo padded blocks
    xb = np.zeros((B, UB, BK))
    xb[:, 1:257, :704] = x88.reshape(B, T, 704) * SX
    xhi = _fp8(xb)
    xlo = _fp8(xb - xhi.astype(np.float64))
    xs = np.stack([xhi, xlo], axis=1)        # [B, 2, UB, BK]
    xh = np.ascontiguousarray(
        xs.reshape(B // 2, 2, 2, UB, KC, 128).transpose(0, 5, 1, 2, 3, 4)
    ).reshape(B // 2, 128, 2, 2 * XC)

    return wsb, xh


def _host_tail(C, x88):
    """Exact host computation of output channels MD..66 plus the bias
    terms (all-zero for the given inputs, kept for generality)."""
    wint, wl, wr = C["wint"], C["wl"], C["wr"]
    xp = np.zeros((B, F + 16, CIN))
    xp[:, 7:7 + F] = x88
    h2 = np.zeros((B, T, COUT - MD))
    for m in range(NTAP):
        h2 += xp[:, m:m + 8 * T:8] @ wint[m, MD:COUT].T
    dwl = wl[:3] - wint[7:10]
    dwr = wr[12:15] - wint[12:15]
    for e in range(ND):
        h2[:, 0] += x88[:, e] @ dwl[e, MD:COUT].T
        h2[:, T - 1] += x88[:, F - ND + e] @ dwr[e, MD:COUT].T
    h2 += C["bint"][MD:COUT]
    h2[:, 0] += (C["bl"] - C["bint"])[MD:COUT]
    h2[:, T - 1] += (C["br"] - C["bint"])[MD:COUT]
    return h2


# ---------------------------------------------------------------------------
# entry point
# ---------------------------------------------------------------------------

def _kernel_impl(**inputs):
    from concourse.bass_utils import run_bass_kernel_spmd

    P = {k: np.asarray(v) for k, v in inputs.items()}
    inp = P.pop("input").astype(np.float64, copy=False)
    off = P.pop("offset").astype(np.float64, copy=False)
    x88 = np.concatenate([inp, off], -1).reshape(B, F, CIN)

    C = _compose(P)
    wsb, xh = _host_pack(C, x88)
    h2 = _host_tail(C, x88)

    in_maps = []
    for c in range(NCORES):
        in_maps.append({
            "wsb": wsb,
            "xh": xh[c * NPAIR:(c + 1) * NPAIR],
        })

    nc = _get_state()
    res = run_bass_kernel_spmd(nc, in_maps, core_ids=list(range(NCORES)))

    out = np.empty((B, T, COUT), np.float32)
    for c in range(NCORES):
        o = res.results[c]["out"].astype(np.float32)             # [64, BL, 256]
        out[c * BL:(c + 1) * BL, :, :MD] = o.transpose(1, 2, 0)
    # bias + boundary-column edge deltas for the device channels (exact,
    # host-side; the device computes the interior approximation only)
    wint, wl, wr = C["wint"], C["wl"], C["wr"]
    out[:, :, :MD] += C["bint"][:MD]
    out[:, 0, :MD] += (C["bl"] - C["bint"])[:MD]
    out[:, T - 1, :MD] += (C["br"] - C["bint"])[:MD]
    dwl = wl[:3] - wint[7:10]
    dwr = wr[12:15] - wint[12:15]
    for e in range(ND):
        out[:, 0, :MD] += x88[:, e] @ dwl[e, :MD].T
        out[:, T - 1, :MD] += x88[:, F - ND + e] @ dwr[e, :MD].T
    out[:, :, MD:] = h2
    return out.reshape(B, T, J, POS)


def _subproc_main(in_path, out_path):
    with open(in_path, "rb") as f:
        import pickle
        inputs = pickle.load(f)
    np.save(out_path, _kernel_impl(**inputs))


def kernel(**inputs):
    """Entry point. The very first execution of a freshly compiled NEFF
    occasionally kills the device session (NRT_EXEC_UNIT_UNRECOVERABLE);
    a rerun in a fresh process reliably succeeds (the compile cache makes
    it cheap). So: try in-process, fall back to fresh subprocesses."""
    if not _STATE.get("dead"):
        try:
            return _kernel_impl(**inputs)
        except Exception:  # noqa: BLE001
            _STATE["dead"] = True  # this process's device session is gone

    import pickle
    import subprocess
    import tempfile

    kdir = os.path.dirname(os.path.abspath(__file__))
    last_err = None
    for _ in range(3):
        with tempfile.TemporaryDirectory() as td:
            ip = os.path.join(td, "in.pkl")
            op = os.path.join(td, "out.npy")
            with open(ip, "wb") as f:
                pickle.dump({k: np.asarray(v) for k, v in inputs.items()}, f,
                            protocol=4)
            code = (
                "import sys; sys.path.insert(0, {kd!r}); import kernel; "
                "kernel._subproc_main({ip!r}, {op!r})"
            ).format(kd=kdir, ip=ip, op=op)
            r = subprocess.run([sys.executable, "-c", code],
                               capture_output=True, text=True)
            if r.returncode == 0 and os.path.exists(op):
                return np.load(op)
            last_err = r.stderr[-2000:] if r.stderr else f"rc={r.returncode}"
    raise RuntimeError(f"kernel subprocess retries exhausted: {last_err}")
